# revision 1
# baseline (speedup 1.0000x reference)
"""Trainium2 Bass kernel for nn_LoRAElementLinear (MoE-routed per-node linear).

Math (reference):
    delta_w[z] = lora_A[z].T-contracted with lora_B[z] * SCALING     # [OUT, IN]
    W[z]       = (weights[z] + delta_w[z]) * ALPHA                   # [OUT, IN]
    out[b]     = sum_z node_attrs[b, z] * (W[z] @ t[b])              # [OUT, M]

node_attrs is a one-hot expert indicator (moe_routing), so out[b] = W[expert(b)] @ t[b].

Sharding strategy (host side): group nodes by expert. With Z=10 experts and 8
cores, pad every expert group to `cap` slots (multiple of 128). Eight experts
("A" experts) are assigned whole to one core each; the remaining two ("B"
experts) are split into 4 quarter-pieces each, one piece per core. Every core
therefore processes exactly NS = cap + cap/4 node slots in two statically-sized
segments — a structurally identical (SPMD) program on all 8 cores.

Per-core HW kernel:
    1. LoRA merge on TensorE:  w[e] = wt[e] + la[e].T @ lbt[e]
       (host pre-scales wt by ALPHA and lbt by SCALING*ALPHA, and pre-transposes
       both to the lhsT [IN, OUT] layout — layout/constant prep only).
    2. Main matmuls: out[:, cols] = w[e].T @ tk[:, cols] streamed in free-dim
       chunks of <=512 columns, PSUM-accumulated over the 4 K-tiles of IN=512.
"""

import os
from math import ceil, sqrt

import numpy as np

import concourse.bass as bass  # noqa: F401  (engine API namespace)
import concourse.mybir as mybir
import concourse.tile as tile
from concourse import bacc
from concourse.bass import ts
from concourse.bass_utils import run_bass_kernel_spmd

B, Z, IN_DIM, OUT_DIM, R, M = 8192, 10, 512, 512, 8, 3
LORA_ALPHA = 8.0
SCALING = LORA_ALPHA / R
ALPHA = 1.0 / sqrt(IN_DIM)
N_CORES = 8
P = 128
KT = IN_DIM // P   # K tiles of the contraction dim
MT = OUT_DIM // P  # output-channel tiles
F32 = mybir.dt.float32
# float32r: single-pass fp32 matmul (4x PE rate vs fp32's 2-pass emulation) at
# tf32-like operand rounding — measured 1.6e-4 rel err vs 1.6e-7 for fp32.
# Set to F32 to trade ~40us for exact fp32 precision.
MM_DT = mybir.dt.float32r

LAST_EXEC_NS = None
LAST_RESULTS = None

_program_cache: dict[int, object] = {}


def _chunk_plan(cap: int, quarter: int):
    """Column chunks [(segment e, col0, ncols)] covering both segments.

    Slots are split into near-even pieces so every chunk is <=512 columns
    (one PSUM bank of fp32)."""
    chunks = []
    for e, slot0, nslots in ((0, 0, cap), (1, cap, quarter)):
        n = max(1, ceil(nslots * 3 / 512))
        # even slot counts => even column counts (f32r matmul needs an even
        # moving free dim)
        base = (nslots // n) & ~1
        sizes = [base] * n
        rem = nslots - base * n
        i = 0
        while rem > 0:
            sizes[i % n] += 2
            rem -= 2
            i += 1
        s = slot0
        for sz in sizes:
            if sz == 0:
                continue
            assert sz * 3 <= 512
            chunks.append((e, s * 3, sz * 3))
            s += sz
    return chunks


def _build_program(cap: int):
    quarter = cap // 4
    ns3 = (cap + quarter) * 3

    nc = bacc.Bacc("TRN2", target_bir_lowering=False, debug=False,
                   num_devices=N_CORES)
    tk_d = nc.dram_tensor("tk", [IN_DIM, ns3], MM_DT, kind="ExternalInput")
    wt_d = nc.dram_tensor("wt", [2, IN_DIM, OUT_DIM], MM_DT, kind="ExternalInput")
    la_d = nc.dram_tensor("la", [2, R, IN_DIM], MM_DT, kind="ExternalInput")
    lbt_d = nc.dram_tensor("lbt", [2, R, OUT_DIM], MM_DT, kind="ExternalInput")
    out_d = nc.dram_tensor("out", [OUT_DIM, ns3], F32, kind="ExternalOutput")

    # [p, kt, c] views: row (kt*128+p) of the [512, ns3] DRAM tensors
    tk_v = tk_d.rearrange("(kt p) c -> p kt c", p=P)
    out_v = out_d.rearrange("(mt p) c -> p mt c", p=P)
    wt_v = wt_d.rearrange("e (kt p) o -> e p kt o", p=P)

    with tile.TileContext(nc) as tc:
        with (
            tc.tile_pool(name="wpool", bufs=1) as wpool,
            tc.tile_pool(name="lpool", bufs=1) as lpool,
            tc.tile_pool(name="psd", bufs=2, space="PSUM") as psd_pool,
            tc.tile_pool(name="tpool", bufs=4) as tpool,
            tc.tile_pool(name="opool", bufs=4) as opool,
            tc.tile_pool(name="pmain", bufs=6, space="PSUM") as pm_pool,
        ):
            # ---- LoRA merge: w_sb[e][:, kt, :] = wt[e, kt] + la[e][:, kt].T @ lbt[e]
            w_sb = {}
            for e in range(2):
                la_sb = lpool.tile([R, IN_DIM], MM_DT, tag=f"la{e}", name=f"la{e}")
                lbt_sb = lpool.tile([R, OUT_DIM], MM_DT, tag=f"lbt{e}",
                                    name=f"lbt{e}")
                nc.sync.dma_start(la_sb[:], la_d[e])
                nc.sync.dma_start(lbt_sb[:], lbt_d[e])
                w = wpool.tile([P, KT, OUT_DIM], MM_DT, tag=f"w{e}", name=f"w{e}")
                nc.sync.dma_start(w[:], wt_v[e])  # one 1 MiB DMA per expert
                for kt in range(KT):
                    pd = psd_pool.tile([P, OUT_DIM], F32, tag="pd",
                                       name=f"pd{e}_{kt}")
                    nc.tensor.matmul(pd[:], la_sb[:, ts(kt, P)], lbt_sb[:],
                                     start=True, stop=True)
                    nc.vector.tensor_add(w[:, kt, :], w[:, kt, :], pd[:])
                w_sb[e] = w

            # ---- main: psum[mt] = sum_kt w[e][:, kt, mt*128:].T @ tin[:, kt, :]
            for e, col0, ncols in _chunk_plan(cap, quarter):
                tin = tpool.tile([P, KT, ncols], MM_DT, tag="tin",
                                 name=f"t_{col0}")
                nc.sync.dma_start(tin[:], tk_v[:, :, col0:col0 + ncols])
                ot = opool.tile([P, MT, ncols], F32, tag="ot", name=f"o_{col0}")
                for mt in range(MT):
                    ps = pm_pool.tile([P, ncols], F32, tag="pm",
                                      name=f"ps_{col0}_{mt}")
                    for kt in range(KT):
                        nc.tensor.matmul(ps[:], w_sb[e][:, kt, ts(mt, P)],
                                         tin[:, kt, :],
                                         start=(kt == 0), stop=(kt == KT - 1))
                    nc.vector.tensor_copy(ot[:, mt, :], ps[:])
                nc.sync.dma_start(out_v[:, :, col0:col0 + ncols], ot[:])

    nc.compile()
    return nc


def _get_program(cap: int):
    if cap not in _program_cache:
        _program_cache[cap] = _build_program(cap)
    return _program_cache[cap]


def _dense_fallback(t, node_attrs, weights, lora_A, lora_B):
    # Host-side general path: only reached if node_attrs is not one-hot
    # (never happens for this problem's setup_inputs).
    delta = np.einsum("zri,zor->zoi", lora_A, lora_B) * SCALING
    W = (weights + delta) * ALPHA
    out = np.zeros((B, OUT_DIM, M), np.float32)
    for z in range(Z):
        out += node_attrs[:, z, None, None] * np.matmul(W[z], t)
    return out


def prepare(t, node_attrs, weights, lora_A, lora_B):
    """Host-side sharding: returns (cap, in_maps, core_nodes) or None if the
    routing matrix is not one-hot (dense fallback needed)."""
    idx = node_attrs.argmax(axis=1)
    onehot = (np.count_nonzero(node_attrs, axis=1) == 1).all() and (
        node_attrs[np.arange(B), idx] == 1.0
    ).all()
    if not onehot:
        return None

    counts = np.bincount(idx, minlength=Z)
    # cap: >= largest expert group; divisible by 8 so quarter-pieces stay even
    cap = max(32, int(ceil(counts.max() / 8)) * 8)
    quarter = cap // 4
    ns3 = (cap + quarter) * 3
    bexp = np.argsort(counts, kind="stable")[:2].tolist()  # the two split experts
    aexp = [z for z in range(Z) if z not in bexp]          # eight whole experts
    nodes_by_z = [np.where(idx == z)[0] for z in range(Z)]

    wt_all = np.ascontiguousarray(weights.transpose(0, 2, 1)) * np.float32(ALPHA)
    lbt_all = np.ascontiguousarray(lora_B.transpose(0, 2, 1)) * np.float32(
        SCALING * ALPHA
    )

    in_maps = []
    core_nodes = []
    for k in range(N_CORES):
        eA = aexp[k]
        eB = bexp[0] if k < 4 else bexp[1]
        piece = k % 4
        nA = nodes_by_z[eA]
        nB = nodes_by_z[eB][piece * quarter:(piece + 1) * quarter]
        tk = np.zeros((IN_DIM, ns3), np.float32)
        if len(nA):
            tk[:, :len(nA) * 3] = t[nA].transpose(1, 0, 2).reshape(IN_DIM, -1)
        if len(nB):
            tk[:, cap * 3:cap * 3 + len(nB) * 3] = (
                t[nB].transpose(1, 0, 2).reshape(IN_DIM, -1)
            )
        in_maps.append({
            "tk": tk,
            "wt": np.ascontiguousarray(wt_all[[eA, eB]]),
            "la": np.ascontiguousarray(lora_A[[eA, eB]]),
            "lbt": np.ascontiguousarray(lbt_all[[eA, eB]]),
        })
        core_nodes.append((nA, nB))
    return cap, in_maps, core_nodes


def assemble(cap, core_nodes, results):
    out_full = np.zeros((B, OUT_DIM, M), np.float32)
    for k in range(N_CORES):
        nA, nB = core_nodes[k]
        o = results[k]["out"]
        if len(nA):
            out_full[nA] = (
                o[:, :len(nA) * 3].reshape(OUT_DIM, len(nA), 3).transpose(1, 0, 2)
            )
        if len(nB):
            out_full[nB] = (
                o[:, cap * 3:cap * 3 + len(nB) * 3]
                .reshape(OUT_DIM, len(nB), 3)
                .transpose(1, 0, 2)
            )
    return out_full


def kernel(t, node_attrs, weights, lora_A, lora_B):
    global LAST_EXEC_NS, LAST_RESULTS
    t = np.ascontiguousarray(t, dtype=np.float32)
    node_attrs = np.asarray(node_attrs, dtype=np.float32)
    weights = np.asarray(weights, dtype=np.float32)
    lora_A = np.ascontiguousarray(lora_A, dtype=np.float32)
    lora_B = np.asarray(lora_B, dtype=np.float32)

    prep = prepare(t, node_attrs, weights, lora_A, lora_B)
    if prep is None:
        return _dense_fallback(t, node_attrs, weights, lora_A, lora_B)
    cap, in_maps, core_nodes = prep

    nc = _get_program(cap)
    res = run_bass_kernel_spmd(nc, in_maps, list(range(N_CORES)))
    LAST_EXEC_NS = res.exec_time_ns
    LAST_RESULTS = res
    return assemble(cap, core_nodes, res.results)



# revision 2
# speedup vs baseline: 160.1345x; 160.1345x over previous
"""Trainium2 Bass kernel for nn_LoRAElementLinear (MoE-routed per-node linear).

Math (reference):
    delta_w[z] = lora_A[z].T-contracted with lora_B[z] * SCALING     # [OUT, IN]
    W[z]       = (weights[z] + delta_w[z]) * ALPHA                   # [OUT, IN]
    out[b]     = sum_z node_attrs[b, z] * (W[z] @ t[b])              # [OUT, M]

node_attrs is a one-hot expert indicator (moe_routing), so out[b] = W[expert(b)] @ t[b].

Sharding strategy (host side): group nodes by expert. With Z=10 experts and 8
cores, pad every expert group to `cap` slots (multiple of 8). Eight experts
("A" experts) are assigned whole to one core each; the remaining two ("B"
experts) are split into 4 quarter-pieces each, one piece per core. Every core
therefore processes exactly NS = cap + cap/4 node slots in two statically-sized
segments — a structurally identical (SPMD) program on all 8 cores.

All HBM streams are bfloat16 (t, weights, LoRA factors, output) — the kernel
is DMA-bound in fp32, and the 2e-2 rel-err budget leaves bf16's ~3e-3 error
comfortable. PSUM accumulation stays fp32. Host-side layouts are pre-swizzled
chunk-major so every DMA moves >=2.5 KB contiguous per partition line.

Per-core HW kernel:
    1. LoRA merge on TensorE:  w[e] = wt[e] + la[e].T @ lbt[e]
       (host pre-scales wt by ALPHA and lbt by SCALING*ALPHA, and pre-transposes
       both to the lhsT [IN, OUT] layout — layout/constant prep only).
    2. Main matmuls: for each column chunk (<=512 cols), psum[mt] accumulates
       over the 4 K-tiles of IN=512; DVE casts psum to bf16; DMA out.
"""

import os
from math import ceil, sqrt

import ml_dtypes
import numpy as np

import concourse.bass as bass  # noqa: F401  (engine API namespace)
import concourse.mybir as mybir
import concourse.tile as tile
from concourse import bacc
from concourse.bass_utils import run_bass_kernel_spmd

B, Z, IN_DIM, OUT_DIM, R, M = 8192, 10, 512, 512, 8, 3
LORA_ALPHA = 8.0
SCALING = LORA_ALPHA / R
ALPHA = 1.0 / sqrt(IN_DIM)
N_CORES = 8
P = 128
KT = IN_DIM // P   # K tiles of the contraction dim
MT = OUT_DIM // P  # output-channel tiles
F32 = mybir.dt.float32
BF16 = mybir.dt.bfloat16
NP_BF16 = ml_dtypes.bfloat16

LAST_EXEC_NS = None
LAST_RESULTS = None

_program_cache: dict[int, object] = {}


def _chunk_plan(cap: int, quarter: int):
    """Column chunks [(segment e, col0, ncols)] covering both segments.

    Slots are split into near-even pieces so every chunk is <=512 columns
    (one PSUM bank of fp32). Chunk column counts are kept even."""
    chunks = []
    for e, slot0, nslots in ((0, 0, cap), (1, cap, quarter)):
        n = max(1, ceil(nslots * 3 / 512))
        base = (nslots // n) & ~1
        sizes = [base] * n
        rem = nslots - base * n
        i = 0
        while rem > 0:
            sizes[i % n] += 2
            rem -= 2
            i += 1
        s = slot0
        for sz in sizes:
            if sz == 0:
                continue
            assert sz * 3 <= 512
            chunks.append((e, s * 3, sz * 3))
            s += sz
    return chunks


def _build_program(cap: int):
    quarter = cap // 4
    ns3 = (cap + quarter) * 3
    totc = KT * ns3   # flat per-partition column count of the swizzled tk

    nc = bacc.Bacc("TRN2", target_bir_lowering=False, debug=False,
                   num_devices=N_CORES)
    tk_d = nc.dram_tensor("tk", [P, totc], BF16, kind="ExternalInput")
    wt_d = nc.dram_tensor("wt", [2, P, KT, OUT_DIM], BF16, kind="ExternalInput")
    la_d = nc.dram_tensor("la", [2, R, IN_DIM], BF16, kind="ExternalInput")
    lbt_d = nc.dram_tensor("lbt", [2, R, OUT_DIM], BF16, kind="ExternalInput")
    out_d = nc.dram_tensor("out", [P, MT * ns3], BF16, kind="ExternalOutput")

    with tile.TileContext(nc) as tc:
        with (
            tc.tile_pool(name="wpool", bufs=1) as wpool,
            tc.tile_pool(name="lpool", bufs=1) as lpool,
            tc.tile_pool(name="psd", bufs=2, space="PSUM") as psd_pool,
            tc.tile_pool(name="tpool", bufs=4) as tpool,
            tc.tile_pool(name="opool", bufs=4) as opool,
            tc.tile_pool(name="pmain", bufs=6, space="PSUM") as pm_pool,
        ):
            # ---- LoRA merge: w_sb[e][:, kt, :] = wt[e, kt] + la[e][:, kt].T @ lbt[e]
            w_sb = {}
            for e in range(2):
                la_sb = lpool.tile([R, IN_DIM], BF16, tag=f"la{e}", name=f"la{e}")
                lbt_sb = lpool.tile([R, OUT_DIM], BF16, tag=f"lbt{e}",
                                    name=f"lbt{e}")
                nc.sync.dma_start(la_sb[:], la_d[e])
                nc.sync.dma_start(lbt_sb[:], lbt_d[e])
                w = wpool.tile([P, KT, OUT_DIM], BF16, tag=f"w{e}", name=f"w{e}")
                nc.sync.dma_start(w[:], wt_d[e])  # one 0.5 MiB DMA per expert
                for kt in range(KT):
                    pd = psd_pool.tile([P, OUT_DIM], F32, tag="pd",
                                       name=f"pd{e}_{kt}")
                    nc.tensor.matmul(pd[:], la_sb[:, kt * P:(kt + 1) * P],
                                     lbt_sb[:], start=True, stop=True)
                    nc.vector.tensor_add(w[:, kt, :], w[:, kt, :], pd[:])
                w_sb[e] = w

            # ---- main: psum[mt] = sum_kt w[e][:, kt, mt*128:].T @ tin[:, kt-slice]
            tkoff = 0   # running flat column offset into tk_d (chunk-major)
            ooff = 0    # running flat column offset into out_d
            for e, col0, ncols in _chunk_plan(cap, quarter):
                tin = tpool.tile([P, KT * ncols], BF16, tag="tin",
                                 name=f"t_{col0}")
                nc.sync.dma_start(tin[:], tk_d[:, tkoff:tkoff + KT * ncols])
                ot = opool.tile([P, MT * ncols], BF16, tag="ot",
                                name=f"o_{col0}")
                for mt in range(MT):
                    ps = pm_pool.tile([P, ncols], F32, tag="pm",
                                      name=f"ps_{col0}_{mt}")
                    for kt in range(KT):
                        nc.tensor.matmul(
                            ps[:],
                            w_sb[e][:, kt, mt * P:(mt + 1) * P],
                            tin[:, kt * ncols:(kt + 1) * ncols],
                            start=(kt == 0), stop=(kt == KT - 1))
                    nc.vector.tensor_copy(ot[:, mt * ncols:(mt + 1) * ncols],
                                          ps[:])
                nc.sync.dma_start(out_d[:, ooff:ooff + MT * ncols], ot[:])
                tkoff += KT * ncols
                ooff += MT * ncols

    nc.compile()
    return nc


def _get_program(cap: int):
    if cap not in _program_cache:
        _program_cache[cap] = _build_program(cap)
    return _program_cache[cap]


def _dense_fallback(t, node_attrs, weights, lora_A, lora_B):
    # Host-side general path: only reached if node_attrs is not one-hot
    # (never happens for this problem's setup_inputs).
    delta = np.einsum("zri,zor->zoi", lora_A, lora_B) * SCALING
    W = (weights + delta) * ALPHA
    out = np.zeros((B, OUT_DIM, M), np.float32)
    for z in range(Z):
        out += node_attrs[:, z, None, None] * np.matmul(W[z], t)
    return out


def prepare(t, node_attrs, weights, lora_A, lora_B):
    """Host-side sharding: returns (cap, in_maps, core_nodes) or None if the
    routing matrix is not one-hot (dense fallback needed)."""
    idx = node_attrs.argmax(axis=1)
    onehot = (np.count_nonzero(node_attrs, axis=1) == 1).all() and (
        node_attrs[np.arange(B), idx] == 1.0
    ).all()
    if not onehot:
        return None

    counts = np.bincount(idx, minlength=Z)
    # cap: >= largest expert group; divisible by 8 so quarter-pieces stay even
    cap = max(32, int(ceil(counts.max() / 8)) * 8)
    quarter = cap // 4
    ns3 = (cap + quarter) * 3
    chunks = _chunk_plan(cap, quarter)
    bexp = np.argsort(counts, kind="stable")[:2].tolist()  # the two split experts
    aexp = [z for z in range(Z) if z not in bexp]          # eight whole experts
    nodes_by_z = [np.where(idx == z)[0] for z in range(Z)]

    # [Z, P, KT, OUT] bf16 stationary layout: row kt*128+p of W[z].T, pre-scaled
    wt_all = np.ascontiguousarray(
        (weights.transpose(0, 2, 1) * np.float32(ALPHA))
        .reshape(Z, KT, P, OUT_DIM)
        .transpose(0, 2, 1, 3)
    ).astype(NP_BF16)
    la_all = lora_A.astype(NP_BF16)
    lbt_all = np.ascontiguousarray(
        lora_B.transpose(0, 2, 1) * np.float32(SCALING * ALPHA)
    ).astype(NP_BF16)

    in_maps = []
    core_nodes = []
    for k in range(N_CORES):
        eA = aexp[k]
        eB = bexp[0] if k < 4 else bexp[1]
        piece = k % 4
        nA = nodes_by_z[eA]
        nB = nodes_by_z[eB][piece * quarter:(piece + 1) * quarter]
        tk = np.zeros((IN_DIM, ns3), np.float32)
        if len(nA):
            tk[:, :len(nA) * 3] = t[nA].transpose(1, 0, 2).reshape(IN_DIM, -1)
        if len(nB):
            tk[:, cap * 3:cap * 3 + len(nB) * 3] = (
                t[nB].transpose(1, 0, 2).reshape(IN_DIM, -1)
            )
        # swizzle to chunk-major [P, sum_chunks(KT*ncols)]: per chunk the
        # per-partition line is KT*ncols contiguous bf16 (>=2.5 KB DMA lines)
        v = tk.reshape(KT, P, ns3).transpose(1, 0, 2)  # [P, KT, ns3]
        tk_sw = np.concatenate(
            [v[:, :, c0:c0 + ncl].reshape(P, KT * ncl) for _, c0, ncl in chunks],
            axis=1,
        ).astype(NP_BF16)
        in_maps.append({
            "tk": tk_sw,
            "wt": np.ascontiguousarray(wt_all[[eA, eB]]),
            "la": np.ascontiguousarray(la_all[[eA, eB]]),
            "lbt": np.ascontiguousarray(lbt_all[[eA, eB]]),
        })
        core_nodes.append((nA, nB))
    return cap, in_maps, core_nodes


def assemble(cap, core_nodes, results):
    quarter = cap // 4
    ns3 = (cap + quarter) * 3
    chunks = _chunk_plan(cap, quarter)
    out_full = np.zeros((B, OUT_DIM, M), np.float32)
    for k in range(N_CORES):
        nA, nB = core_nodes[k]
        o = results[k]["out"]  # [P, MT*ns3] bf16, chunk-major
        ofull = np.empty((OUT_DIM, ns3), np.float32)
        ooff = 0
        for _, c0, ncl in chunks:
            blk = o[:, ooff:ooff + MT * ncl].reshape(P, MT, ncl)
            ofull[:, c0:c0 + ncl] = (
                blk.transpose(1, 0, 2).reshape(OUT_DIM, ncl)
            )
            ooff += MT * ncl
        if len(nA):
            out_full[nA] = (
                ofull[:, :len(nA) * 3]
                .reshape(OUT_DIM, len(nA), 3)
                .transpose(1, 0, 2)
            )
        if len(nB):
            out_full[nB] = (
                ofull[:, cap * 3:cap * 3 + len(nB) * 3]
                .reshape(OUT_DIM, len(nB), 3)
                .transpose(1, 0, 2)
            )
    return out_full


def kernel(t, node_attrs, weights, lora_A, lora_B):
    global LAST_EXEC_NS, LAST_RESULTS
    t = np.ascontiguousarray(t, dtype=np.float32)
    node_attrs = np.asarray(node_attrs, dtype=np.float32)
    weights = np.asarray(weights, dtype=np.float32)
    lora_A = np.ascontiguousarray(lora_A, dtype=np.float32)
    lora_B = np.asarray(lora_B, dtype=np.float32)

    prep = prepare(t, node_attrs, weights, lora_A, lora_B)
    if prep is None:
        return _dense_fallback(t, node_attrs, weights, lora_A, lora_B)
    cap, in_maps, core_nodes = prep

    nc = _get_program(cap)
    res = run_bass_kernel_spmd(nc, in_maps, list(range(N_CORES)))
    LAST_EXEC_NS = res.exec_time_ns
    LAST_RESULTS = res
    return assemble(cap, core_nodes, res.results)
